# revision 1
# baseline (speedup 1.0000x reference)
"""DCT-II (unnormalized, along last dim) on 8 trn2 NeuronCores.

y[r, k] = sum_n x[r, n] * cos(pi/N * (n+0.5) * k),  x: [2048, 4096] fp32.

This is y = x @ C^T with C the NxN cosine table. Sharding: the output is
split 2-way along rows x 4-way along frequencies; core c = (rb, kb) computes
y[rb*1024:(rb+1)*1024, kb*1024:(kb+1)*1024] = xT_shard.T @ ct_shard with
xT_shard = x[rows].T  [4096, 1024] and ct_shard = C.T[:, cols] [4096, 1024].

The cosine table is a constant computed on host in fp32 with the same op
order as the reference.
"""

import numpy as np

N = 4096
R = 2048
RB, KB = 2, 4
RS, KS = R // RB, N // KB

# matmul input dtype: "float32" (exact, 4 cyc/row) or "float32r" (1 cyc/row)
MM_DTYPE = "float32r"

_state = {}


def _cos_table_t():
    """C.T[n, k] in fp32, matching the reference's fp32 arithmetic."""
    n = np.arange(N, dtype=np.float32)
    k = np.arange(N, dtype=np.float32)
    a = np.float32(np.pi / N) * (n + np.float32(0.5))  # [N] fp32
    arg = a[:, None] * k[None, :]  # [n, k] fp32
    return np.cos(arg)  # fp32 cos


def _build():
    import concourse.tile as tile
    from concourse import bacc, mybir
    from concourse.kernels.tile_matmul import matmul_tile_kernel

    dt_in = getattr(mybir.dt, MM_DTYPE)
    nc = bacc.Bacc("TRN2", target_bir_lowering=False, debug=False, num_devices=8)
    xt = nc.dram_tensor("xt", [N, RS], dt_in, kind="ExternalInput").ap()
    ct = nc.dram_tensor("ct", [N, KS], dt_in, kind="ExternalInput").ap()
    y = nc.dram_tensor("y", [RS, KS], mybir.dt.float32, kind="ExternalOutput").ap()
    with tile.TileContext(nc) as tc:
        matmul_tile_kernel(tc, xt, ct, y)
    nc.compile()
    return nc


def kernel(x: np.ndarray, _trace: bool = False):
    from concourse.bass_utils import run_bass_kernel_spmd

    assert x.shape == (R, N) and x.dtype == np.float32
    if "nc" not in _state:
        _state["nc"] = _build()
    if "ct" not in _state:
        _state["ct"] = np.ascontiguousarray(_cos_table_t())
    nc = _state["nc"]
    ctt = _state["ct"]

    in_maps = []
    for c in range(8):
        rb, kb = divmod(c, KB)
        in_maps.append(
            {
                "xt": np.ascontiguousarray(x[rb * RS : (rb + 1) * RS, :].T),
                "ct": np.ascontiguousarray(ctt[:, kb * KS : (kb + 1) * KS]),
            }
        )

    res = run_bass_kernel_spmd(nc, in_maps, list(range(8)), trace=_trace)

    y = np.empty((R, N), dtype=np.float32)
    for c in range(8):
        rb, kb = divmod(c, KB)
        y[rb * RS : (rb + 1) * RS, kb * KS : (kb + 1) * KS] = res.results[c]["y"]
    if _trace:
        _state["last_result"] = res
    return y
